# revision 1
# baseline (speedup 1.0000x reference)
"""Multihead causal attention on 8 TRN2 NeuronCores.

Problem: B=4, S=2048, E=1024, H=16 heads, D=64. Causal mask, eval mode.
Sharding: batch x head-group. Core c -> batch b = c//2, head group g = c%2
(8 heads = 512 hidden dims per core). Each core computes QKV projections for
its head group on its batch, causal flash-style attention, and a partial
output projection. Host sums the two partials per batch and adds bo.

Layout strategy (per core):
  - Activations DMA'd pre-transposed from host: qTr/kTr/vTr [E=1024, S=2048].
  - q/k projections in [d, s] layout (qT/kT [128, 2048] per head pair) so the
    scores matmul scoresT[k, q] = kT.T @ qT needs no on-chip transposes and
    biases are per-partition.
  - Scores matmuls (K=64) for the two heads of a pair are issued adjacently
    at base partitions 0/64 -> they run concurrently in separate PE row
    groups (~2x).
  - v projection in natural [s, d] layout, augmented with a ones column ->
    the AV matmul produces softmax denominators for free (row 64 of PSUM).
  - No max-subtraction in softmax: scores ~ N(0,1) by construction.
  - bf16 matmul inputs (fp32 PSUM accumulation); softmax statistics and
    normalization in fp32.
"""
import math
import numpy as np
import ml_dtypes

import concourse.bass as bass
import concourse.mybir as mybir
import concourse.tile as tile
from concourse import bacc
from concourse.bass_utils import run_bass_kernel_spmd

F32 = mybir.dt.float32
BF16 = mybir.dt.bfloat16
AF = mybir.ActivationFunctionType
ALU = mybir.AluOpType

B, S, E, H, D = 4, 2048, 1024, 16, 64
P = 128
NCORES = 8
HPC = 512          # hidden dims per core (8 heads)
NPAIR = 4          # head pairs per core
NSC = S // 512     # 4 s-chunks of 512
NST = S // P       # 16 s-tiles of 128
NQT = S // P       # 16 q-tiles for out proj
NE = E // P        # 8 e-chunks


def _build_nc(debug=False):
    nc = bacc.Bacc(None)
    qTr = nc.declare_dram_parameter("qTr", [E, S], BF16, isOutput=False)
    kTr = nc.declare_dram_parameter("kTr", [E, S], BF16, isOutput=False)
    vTr = nc.declare_dram_parameter("vTr", [E, S], BF16, isOutput=False)
    wq = nc.declare_dram_parameter("wq", [E, HPC], BF16, isOutput=False)
    wk = nc.declare_dram_parameter("wk", [E, HPC], BF16, isOutput=False)
    wv = nc.declare_dram_parameter("wv", [E, HPC], BF16, isOutput=False)
    wo = nc.declare_dram_parameter("wo", [HPC, E], BF16, isOutput=False)
    bq = nc.declare_dram_parameter("bq", [P, NPAIR], F32, isOutput=False)
    bk = nc.declare_dram_parameter("bk", [P, NPAIR], F32, isOutput=False)
    bv = nc.declare_dram_parameter("bv", [P, NPAIR], F32, isOutput=False)
    ones = nc.declare_dram_parameter("ones", [P, 1], BF16, isOutput=False)
    out = nc.declare_dram_parameter("out", [S, E], F32, isOutput=True)

    with tile.TileContext(nc) as tc:
        with (
            tc.tile_pool(name="persist", bufs=1) as persist,
            tc.tile_pool(name="onorm", bufs=1) as onorm_pool,
        ):
            # persistent per-pair projection outputs
            qT = [persist.tile([P, S], BF16, tag=f"qT{p}", name=f"qT{p}")
                  for p in range(NPAIR)]
            kT = [persist.tile([P, S], BF16, tag=f"kT{p}", name=f"kT{p}")
                  for p in range(NPAIR)]
            # v_aug[p][st]: [128, 130]; cols 64/129 = ones, 0:64 / 65:129 = v
            v_aug = [[persist.tile([P, 130], BF16, tag=f"v{p}_{st}",
                                   name=f"v{p}_{st}")
                      for st in range(NST)] for p in range(NPAIR)]
            bv_t = persist.tile([P, NPAIR], F32, tag="bv")
            nc.sync.dma_start(out=bv_t[:], in_=bv[:, :])

            # ---------------- Phase 1: projections ----------------
            with (
                tc.tile_pool(name="weights", bufs=1) as wpool,
                tc.tile_pool(name="acts", bufs=3) as apool,
                tc.tile_pool(name="psum1", bufs=1, space="PSUM") as ps1,
            ):
                wq_t = [wpool.tile([P, HPC], BF16, tag=f"wq{e}", name=f"wq{e}")
                        for e in range(NE)]
                wk_t = [wpool.tile([P, HPC], BF16, tag=f"wk{e}", name=f"wk{e}")
                        for e in range(NE)]
                wv_t = [wpool.tile([P, HPC], BF16, tag=f"wv{e}", name=f"wv{e}")
                        for e in range(NE)]
                for e in range(NE):
                    nc.sync.dma_start(out=wq_t[e][:], in_=wq[e * P:(e + 1) * P, :])
                    nc.sync.dma_start(out=wk_t[e][:], in_=wk[e * P:(e + 1) * P, :])
                    nc.sync.dma_start(out=wv_t[e][:], in_=wv[e * P:(e + 1) * P, :])
                bq_t = wpool.tile([P, NPAIR], F32, tag="bq")
                bk_t = wpool.tile([P, NPAIR], F32, tag="bk")
                nc.sync.dma_start(out=bq_t[:], in_=bq[:, :])
                nc.sync.dma_start(out=bk_t[:], in_=bk[:, :])
                ones_src = wpool.tile([P, 1], BF16, tag="ones")
                nc.sync.dma_start(out=ones_src[:], in_=ones[:, :])
                for p_ in range(NPAIR):
                    for st in range(NST):
                        nc.vector.tensor_copy(v_aug[p_][st][:, 64:65], ones_src[:])
                        nc.vector.tensor_copy(v_aug[p_][st][:, 129:130], ones_src[:])

                for sc in range(NSC):
                    cs = slice(sc * 512, (sc + 1) * 512)
                    q_ps = [ps1.tile([P, 512], F32, tag=f"p1_{m}", name=f"qps{m}")
                            for m in range(NPAIR)]
                    k_ps = [ps1.tile([P, 512], F32, tag=f"p1_{4 + m}", name=f"kps{m}")
                            for m in range(NPAIR)]
                    for e in range(NE):
                        qtr_e = apool.tile([P, 512], BF16, tag="qtr")
                        ktr_e = apool.tile([P, 512], BF16, tag="ktr")
                        nc.sync.dma_start(out=qtr_e[:], in_=qTr[e * P:(e + 1) * P, cs])
                        nc.sync.dma_start(out=ktr_e[:], in_=kTr[e * P:(e + 1) * P, cs])
                        for m in range(NPAIR):
                            ws = slice(m * P, (m + 1) * P)
                            nc.tensor.matmul(q_ps[m][:], wq_t[e][:, ws], qtr_e[:],
                                             start=(e == 0), stop=(e == NE - 1))
                            nc.tensor.matmul(k_ps[m][:], wk_t[e][:, ws], ktr_e[:],
                                             start=(e == 0), stop=(e == NE - 1))
                    for m in range(NPAIR):
                        nc.vector.tensor_scalar(
                            out=qT[m][:, cs], in0=q_ps[m][:],
                            scalar1=bq_t[:, m:m + 1], scalar2=None, op0=ALU.add)
                        nc.vector.tensor_scalar(
                            out=kT[m][:, cs], in0=k_ps[m][:],
                            scalar1=bk_t[:, m:m + 1], scalar2=None, op0=ALU.add)
                    # v projection: out [s, hd] for the 4 s-tiles of this chunk
                    v_ps = [ps1.tile([P, HPC], F32, tag=f"p1_{i}", name=f"vps{i}")
                            for i in range(4)]
                    for e in range(NE):
                        vtr_e = apool.tile([P, 512], BF16, tag="vtr")
                        nc.sync.dma_start(out=vtr_e[:], in_=vTr[e * P:(e + 1) * P, cs])
                        for i in range(4):
                            nc.tensor.matmul(
                                v_ps[i][:], vtr_e[:, i * P:(i + 1) * P], wv_t[e][:],
                                start=(e == 0), stop=(e == NE - 1))
                    for i in range(4):
                        st = sc * 4 + i
                        for p_ in range(NPAIR):
                            nc.vector.tensor_copy(
                                v_aug[p_][st][:, 0:64],
                                v_ps[i][:, p_ * P:p_ * P + 64])
                            nc.vector.tensor_copy(
                                v_aug[p_][st][:, 65:129],
                                v_ps[i][:, p_ * P + 64:(p_ + 1) * P])

            # ---------------- Phase 2: attention ----------------
            with (
                tc.tile_pool(name="sc_ps", bufs=2, space="PSUM") as sc_pool,
                tc.tile_pool(name="av_ps", bufs=4, space="PSUM") as av_pool,
                tc.tile_pool(name="exp", bufs=4) as exp_pool,
                tc.tile_pool(name="small", bufs=1) as small_pool,
                tc.tile_pool(name="tmp", bufs=2) as tmp_pool,
            ):
                out_norm = []
                for p_ in range(NPAIR):
                    tmp_p = tmp_pool.tile([P, S], F32, tag="tmp")
                    for qh in range(2):
                        av = {}
                        for h01 in range(2):
                            for qcl in range(2):
                                av[(h01, qcl)] = av_pool.tile(
                                    [65, 512], F32, tag="av", name="av")
                        for kt in range(8 * qh + 8):
                            vq = [qcl for qcl in range(2)
                                  if (2 * qh + qcl) >= kt // 4]
                            # scores: both heads adjacent -> concurrent row
                            # groups (base partitions 0 and 64)
                            sc_t = [sc_pool.tile([P, 1024], F32, tag="sc",
                                                 name=f"sct{h}") for h in range(2)]
                            for qcl in vq:
                                qc = 2 * qh + qcl
                                for h01 in range(2):
                                    hsl = slice(h01 * 64, (h01 + 1) * 64)
                                    nc.tensor.matmul(
                                        sc_t[h01][:, qcl * 512:(qcl + 1) * 512],
                                        kT[p_][hsl, kt * P:(kt + 1) * P],
                                        qT[p_][hsl, qc * 512:(qc + 1) * 512],
                                        start=True, stop=True)
                            diag = (kt // 4 >= 2 * qh)
                            qcl_d = kt // 4 - 2 * qh  # valid when diag
                            # fully-masked prefix width inside diagonal block
                            j0 = kt * P - (kt // 4) * 512 if diag else 0
                            exs = []
                            for h01 in range(2):
                                ex = exp_pool.tile([P, 1024], BF16, tag="ex",
                                                   name=f"ex{h01}")
                                off = vq[0] * 512
                                if diag:
                                    eoff = qcl_d * 512 + j0
                                    if eoff > off:
                                        nc.vector.memset(ex[:, off:eoff], 0.0)
                                else:
                                    eoff = off
                                nc.scalar.activation(
                                    ex[:, eoff:1024], sc_t[h01][:, eoff:1024],
                                    AF.Exp, scale=1.0 / math.sqrt(D))
                                if diag:
                                    # mask staircase in the 128 cols at eoff
                                    nc.gpsimd.affine_select(
                                        out=ex[:, eoff:eoff + P],
                                        in_=ex[:, eoff:eoff + P],
                                        compare_op=ALU.is_ge, fill=0.0,
                                        base=0, channel_multiplier=-1,
                                        pattern=[[1, P]])
                                exs.append(ex)
                            for h01 in range(2):
                                for qcl in vq:
                                    qc = 2 * qh + qcl
                                    nc.tensor.matmul(
                                        av[(h01, qcl)][:],
                                        v_aug[p_][kt][:, h01 * 65:(h01 + 1) * 65],
                                        exs[h01][:, qcl * 512:(qcl + 1) * 512],
                                        start=(kt == 0), stop=(kt == qc * 4 + 3))
                        # normalization for this q-half
                        sums_h = [small_pool.tile([1, 1024], F32, tag=f"sums{h}",
                                                  name=f"sums{h}")
                                  for h in range(2)]
                        for h01 in range(2):
                            for qcl in range(2):
                                nc.vector.tensor_copy(
                                    sums_h[h01][0:1, qcl * 512:(qcl + 1) * 512],
                                    av[(h01, qcl)][64:65, :])
                        bc_h = [small_pool.tile([64, 1024], F32, tag=f"bc{h}",
                                                name=f"bc{h}") for h in range(2)]
                        for h01 in range(2):
                            nc.gpsimd.partition_broadcast(
                                bc_h[h01][:], sums_h[h01][0:1, :], channels=64)
                            nc.vector.reciprocal_approx_fast(
                                bc_h[h01][:], bc_h[h01][:])
                        for h01 in range(2):
                            for qcl in range(2):
                                qc = 2 * qh + qcl
                                nc.vector.tensor_tensor(
                                    out=tmp_p[h01 * 64:(h01 + 1) * 64,
                                              qc * 512:(qc + 1) * 512],
                                    in0=av[(h01, qcl)][0:64, :],
                                    in1=bc_h[h01][:, qcl * 512:(qcl + 1) * 512],
                                    op=ALU.mult)
                    out_norm_p = onorm_pool.tile([P, S], BF16, tag=f"on{p_}",
                                                 name=f"on{p_}")
                    nc.vector.tensor_scalar(
                        out=out_norm_p[:], in0=tmp_p[:],
                        scalar1=bv_t[:, p_:p_ + 1], scalar2=None, op0=ALU.add)
                    out_norm.append(out_norm_p)

            # ---------------- Phase 3: output projection ----------------
            with (
                tc.tile_pool(name="wo", bufs=1) as wo_pool,
                tc.tile_pool(name="fin", bufs=3) as fin_pool,
                tc.tile_pool(name="psum3", bufs=2, space="PSUM") as ps3,
            ):
                wo_t = [wo_pool.tile([P, E], BF16, tag=f"wo{p}", name=f"wo{p}")
                        for p in range(NPAIR)]
                for p_ in range(NPAIR):
                    nc.sync.dma_start(out=wo_t[p_][:], in_=wo[p_ * P:(p_ + 1) * P, :])
                for qt in range(NQT):
                    fin = fin_pool.tile([P, E], F32, tag="fin")
                    for ec in range(2):
                        ops = ps3.tile([P, 512], F32, tag="o3")
                        for p_ in range(NPAIR):
                            nc.tensor.matmul(
                                ops[:],
                                out_norm[p_][:, qt * P:(qt + 1) * P],
                                wo_t[p_][:, ec * 512:(ec + 1) * 512],
                                start=(p_ == 0), stop=(p_ == NPAIR - 1))
                        nc.vector.tensor_copy(fin[:, ec * 512:(ec + 1) * 512], ops[:])
                    nc.sync.dma_start(out=out[qt * P:(qt + 1) * P, :], in_=fin[:])
    nc.finalize()
    return nc


_NC_CACHE = None


def _get_nc():
    global _NC_CACHE
    if _NC_CACHE is None:
        _NC_CACHE = _build_nc()
    return _NC_CACHE


def _bf(x):
    return np.ascontiguousarray(np.asarray(x, np.float32)).astype(
        ml_dtypes.bfloat16)


def _prepare_in_maps(query, key, value, Wq, bq, Wk, bk, Wv, bv, Wo):
    qTr = [_bf(query[b].T) for b in range(B)]
    kTr = [_bf(key[b].T) for b in range(B)]
    vTr = [_bf(value[b].T) for b in range(B)]
    ones = np.ones((P, 1), ml_dtypes.bfloat16)

    def wslice(Wx, g):
        return _bf(Wx[g * HPC:(g + 1) * HPC, :].T)

    def bslice(bx, g):
        return np.ascontiguousarray(
            np.asarray(bx, np.float32)[g * HPC:(g + 1) * HPC]
            .reshape(NPAIR, P).T)

    wq_g = [wslice(Wq, g) for g in range(2)]
    wk_g = [wslice(Wk, g) for g in range(2)]
    wv_g = [wslice(Wv, g) for g in range(2)]
    wo_g = [_bf(np.asarray(Wo, np.float32)[:, g * HPC:(g + 1) * HPC].T)
            for g in range(2)]
    bq_g = [bslice(bq, g) for g in range(2)]
    bk_g = [bslice(bk, g) for g in range(2)]
    bv_g = [bslice(bv, g) for g in range(2)]

    in_maps = []
    for c in range(NCORES):
        b, g = c // 2, c % 2
        in_maps.append({
            "qTr": qTr[b], "kTr": kTr[b], "vTr": vTr[b],
            "wq": wq_g[g], "wk": wk_g[g], "wv": wv_g[g], "wo": wo_g[g],
            "bq": bq_g[g], "bk": bk_g[g], "bv": bv_g[g], "ones": ones,
        })
    return in_maps


def kernel(query, key, value, attn_mask, Wq, bq, Wk, bk, Wv, bv, Wo, bo,
           _want_timing=False):
    in_maps = _prepare_in_maps(query, key, value, Wq, bq, Wk, bk, Wv, bv, Wo)
    nc = _get_nc()
    res = run_bass_kernel_spmd(nc, in_maps, list(range(NCORES)),
                               trace=bool(_want_timing))
    bo = np.asarray(bo, np.float32)
    out = np.empty((B, S, E), np.float32)
    for b in range(B):
        out[b] = res.results[2 * b]["out"] + res.results[2 * b + 1]["out"] + bo
    if _want_timing:
        return out, res
    return out



# revision 12
# speedup vs baseline: 1.2637x; 1.2637x over previous
"""Multihead causal attention on 8 TRN2 NeuronCores.

Problem: B=4, S=2048, E=1024, H=16 heads, D=64. Causal mask, eval mode.
Sharding: batch x head-group. Core c -> batch b = c//2, head group g = c%2
(8 heads = 512 hidden dims per core). Each core computes QKV projections for
its head group on its batch, causal attention, and a partial output
projection. Host sums the two partials per batch and adds bo.

v2 layout/schedule strategy (per core):
  - Activations DMA'd pre-transposed from host: qTr/kTr/vTr [E=1024, S=2048].
  - q/k projections in [d, s] layout (qT/kT [128, 2048] per head pair) so the
    scores matmul scoresT[k, q] = kT.T @ qT needs no on-chip transposes.
  - Scores matmuls (K=64) for the two heads of a pair are issued adjacently
    at base partitions 0/64 -> concurrent PE row groups.
  - Scores PSUM tiles are bf16 [128, 1024] = ONE bank each (vs 2 for fp32),
    allowing proj/outproj PSUM accumulators to coexist -> projection and
    output-projection matmuls interleave with attention as PE fill while
    the Scalar engine (exp) is the per-tile bottleneck.
  - Causal trimming at 128-col granularity: scores/exp/AV all start at the
    diagonal; no memsets needed since AV never reads below-diagonal cols.
  - v stored per pair as [128, 16*130] with ones at cols 64/129 of each
    130-block -> AV matmul yields softmax denominators for free (row 64).
  - No max-subtraction in softmax: scores ~ N(0,1) by construction.
"""
import math
from collections import deque
import numpy as np
import ml_dtypes

import concourse.bass as bass
import concourse.mybir as mybir
import concourse.tile as tile
from concourse import bacc
from concourse.bass_utils import run_bass_kernel_spmd

F32 = mybir.dt.float32
BF16 = mybir.dt.bfloat16
AF = mybir.ActivationFunctionType
ALU = mybir.AluOpType

B, S, E, H, D = 4, 2048, 1024, 16, 64
P = 128
NCORES = 8
HPC = 512          # hidden dims per core (8 heads)
NPAIR = 4          # head pairs per core
NSC = S // 512     # 4 s-chunks of 512
NST = S // P       # 16 s-tiles of 128
NQT = S // P       # 16 q-tiles for out proj
NE = E // P        # 8 e-chunks


def _build_nc(debug=False):
    nc = bacc.Bacc(None)
    qTr = nc.declare_dram_parameter("qTr", [E, S], BF16, isOutput=False)
    kTr = nc.declare_dram_parameter("kTr", [E, S], BF16, isOutput=False)
    vTr = nc.declare_dram_parameter("vTr", [E, S], BF16, isOutput=False)
    wq = nc.declare_dram_parameter("wq", [E, HPC], BF16, isOutput=False)
    wk = nc.declare_dram_parameter("wk", [E, HPC], BF16, isOutput=False)
    wv = nc.declare_dram_parameter("wv", [E, HPC], BF16, isOutput=False)
    wo = nc.declare_dram_parameter("wo", [HPC, E], BF16, isOutput=False)
    bq = nc.declare_dram_parameter("bq", [P, NPAIR], F32, isOutput=False)
    bk = nc.declare_dram_parameter("bk", [P, NPAIR], F32, isOutput=False)
    bv = nc.declare_dram_parameter("bv", [P, NPAIR], F32, isOutput=False)
    out = nc.declare_dram_parameter("out", [S, E], F32, isOutput=True)

    with tile.TileContext(nc) as tc:
        with (
            tc.tile_pool(name="persist", bufs=1) as persist,
            tc.tile_pool(name="weights", bufs=1) as wpool,
            tc.tile_pool(name="acts", bufs=48) as apool,
            tc.tile_pool(name="ps512", bufs=2, space="PSUM") as ps512,
            tc.tile_pool(name="sc_ps", bufs=4, space="PSUM") as sc_pool,
            tc.tile_pool(name="av_ps", bufs=2, space="PSUM") as av_pool,
            tc.tile_pool(name="exp", bufs=8) as ex_pool,
            tc.tile_pool(name="norm", bufs=1) as npool,
            tc.tile_pool(name="fin", bufs=3) as fin_pool,
        ):
            # ---- persistent tiles ----
            qT = [persist.tile([P, S], BF16, tag=f"qT{p}", name=f"qT{p}")
                  for p in range(NPAIR)]
            kT = [persist.tile([P, S], BF16, tag=f"kT{p}", name=f"kT{p}")
                  for p in range(NPAIR)]
            # v per pair: 16 blocks of 130 cols; cols 64/129 of each block = 1
            v_aug = [persist.tile([P, NST * 130], BF16, tag=f"v{p}",
                                  name=f"v{p}") for p in range(NPAIR)]
            out_norm = [persist.tile([P, S], BF16, tag=f"on{p}", name=f"on{p}")
                        for p in range(NPAIR)]
            bq_t = persist.tile([P, NPAIR], F32, tag="bq")
            bk_t = persist.tile([P, NPAIR], F32, tag="bk")
            bv_t = persist.tile([P, NPAIR], F32, tag="bv")
            nc.sync.dma_start(out=bq_t[:], in_=bq[:, :])
            nc.sync.dma_start(out=bk_t[:], in_=bk[:, :])
            nc.sync.dma_start(out=bv_t[:], in_=bv[:, :])

            # ones columns of v_aug: strided memset (offsets 64+65k)
            for p_ in range(NPAIR):
                nc.gpsimd.memset(v_aug[p_][:, 64:NST * 130:65], 1.0)

            # ---- weights ----
            wq_t = [wpool.tile([P, HPC], BF16, tag=f"wq{e}", name=f"wq{e}")
                    for e in range(NE)]
            wk_t = [wpool.tile([P, HPC], BF16, tag=f"wk{e}", name=f"wk{e}")
                    for e in range(NE)]
            wv_t = [wpool.tile([P, HPC], BF16, tag=f"wv{e}", name=f"wv{e}")
                    for e in range(NE)]
            for e in range(NE):
                nc.sync.dma_start(out=wq_t[e][:], in_=wq[e * P:(e + 1) * P, :])
            for e in range(NE):
                nc.sync.dma_start(out=wk_t[e][:], in_=wk[e * P:(e + 1) * P, :])
            for e in range(NE):
                nc.sync.dma_start(out=wv_t[e][:], in_=wv[e * P:(e + 1) * P, :])
            wo_t = [wpool.tile([P, E], BF16, tag=f"wo{p}", name=f"wo{p}")
                    for p in range(NPAIR)]
            for p_ in range(NPAIR):
                nc.sync.dma_start(out=wo_t[p_][:], in_=wo[p_ * P:(p_ + 1) * P, :])

            # ---- projection work, chunked into fill units ----
            def proj_dma(c):
                cs = slice(c * 512, (c + 1) * 512)
                tiles = {}
                for nm, src in (("q", qTr), ("k", kTr), ("v", vTr)):
                    for e in range(NE):
                        t = apool.tile([P, 512], BF16, tag="act",
                                       name=f"{nm}tr{c}_{e}")
                        nc.sync.dma_start(out=t[:], in_=src[e * P:(e + 1) * P, cs])
                        tiles[(nm, e)] = t
                return tiles

            def proj_qk_unit(c, acts, m, which):
                # one 512-col projection accumulator for pair m (q or k)
                cs = slice(c * 512, (c + 1) * 512)
                w_t, b_t, dstT, nm = {
                    "q": (wq_t, bq_t, qT, "q"), "k": (wk_t, bk_t, kT, "k"),
                }[which]
                ps = ps512.tile([P, 512], F32, tag="ps512", name=f"{nm}ps{c}_{m}")
                ws = slice(m * P, (m + 1) * P)
                for e in range(NE):
                    nc.tensor.matmul(ps[:], w_t[e][:, ws], acts[(nm, e)][:],
                                     start=(e == 0), stop=(e == NE - 1))
                nc.vector.tensor_scalar(
                    out=dstT[m][:, cs], in0=ps[:],
                    scalar1=b_t[:, m:m + 1], scalar2=None, op0=ALU.add)

            def proj_v_unit(c, acts, i):
                # v projection for s-tile i of chunk c: out [s128, 512 hd]
                st = c * 4 + i
                ps = ps512.tile([P, 512], F32, tag="ps512", name=f"vps{c}_{i}")
                for e in range(NE):
                    nc.tensor.matmul(
                        ps[:], acts[("v", e)][:, i * P:(i + 1) * P], wv_t[e][:],
                        start=(e == 0), stop=(e == NE - 1))
                for p_ in range(NPAIR):
                    base = st * 130
                    nc.vector.tensor_copy(
                        v_aug[p_][:, base:base + 64],
                        ps[:, p_ * P:p_ * P + 64])
                    nc.vector.tensor_copy(
                        v_aug[p_][:, base + 65:base + 129],
                        ps[:, p_ * P + 64:(p_ + 1) * P])

            def proj_units(c):
                units = []
                acts = {}
                def dma_unit():
                    acts.update(proj_dma(c))
                units.append(dma_unit)
                for m in range(NPAIR):
                    units.append(lambda m=m: proj_qk_unit(c, acts, m, "q"))
                    units.append(lambda m=m: proj_qk_unit(c, acts, m, "k"))
                for i in range(4):
                    units.append(lambda i=i: proj_v_unit(c, acts, i))
                return units

            # ---- output projection unit (one q-tile) ----
            def outproj_unit(qt):
                fin = fin_pool.tile([P, E], F32, tag="fin")
                for ec in range(2):
                    ops = ps512.tile([P, 512], F32, tag="ps512", name=f"o{qt}_{ec}")
                    for p_ in range(NPAIR):
                        nc.tensor.matmul(
                            ops[:],
                            out_norm[p_][:, qt * P:(qt + 1) * P],
                            wo_t[p_][:, ec * 512:(ec + 1) * 512],
                            start=(p_ == 0), stop=(p_ == NPAIR - 1))
                    nc.vector.tensor_copy(fin[:, ec * 512:(ec + 1) * 512], ops[:])
                nc.sync.dma_start(out=out[qt * P:(qt + 1) * P, :], in_=fin[:])

            # ---- attention for one (pair, q-half), with PE fill units ----
            fill_q = deque()
            debt = [0.0]

            def fill_step():
                while debt[0] > 2000.0 and fill_q:
                    fill_q.popleft()()
                    debt[0] -= 2000.0

            def attn(p_, qc):
                # one pair, one 512-wide q window [qc*512, (qc+1)*512)
                qlo = qc * 512
                nkt = 4 * qc + 4
                av = [av_pool.tile([65, 512], F32, tag="av", name=f"av{h}")
                      for h in range(2)]
                for kt in range(nkt):
                    k0 = kt * P
                    sstart = max(0, k0 - qlo)
                    diag = kt >= 4 * qc
                    W = 512 - sstart
                    sc_t = [sc_pool.tile([P, 512], F32, tag="sc",
                                         name=f"sct{h}") for h in range(2)]
                    for h01 in range(2):
                        hsl = slice(h01 * 64, (h01 + 1) * 64)
                        nc.tensor.matmul(
                            sc_t[h01][:, sstart:512],
                            kT[p_][hsl, k0:k0 + P],
                            qT[p_][hsl, qlo + sstart:qlo + 512],
                            start=True, stop=True)
                    exs = []
                    for h01 in range(2):
                        ex = ex_pool.tile([P, 512], BF16, tag="ex",
                                          name=f"ex{h01}")
                        nc.scalar.activation(
                            ex[:, sstart:512], sc_t[h01][:, sstart:512],
                            AF.Exp, scale=1.0 / math.sqrt(D))
                        if diag:
                            nc.gpsimd.affine_select(
                                out=ex[:, sstart:sstart + P],
                                in_=ex[:, sstart:sstart + P],
                                compare_op=ALU.is_ge, fill=0.0,
                                base=0, channel_multiplier=-1,
                                pattern=[[1, P]])
                        exs.append(ex)
                    for h01 in range(2):
                        vbase = kt * 130 + h01 * 65
                        nc.tensor.matmul(
                            av[h01][:, sstart:512],
                            v_aug[p_][:, vbase:vbase + 65],
                            exs[h01][:, sstart:512],
                            start=(kt == 0), stop=(kt == nkt - 1))
                    # PE/Scalar balance bookkeeping -> drain fill units
                    scalar_ns = 2 * (W + 352) / 1.2
                    pe_ns = 3 * W * 0.48
                    debt[0] += scalar_ns - pe_ns
                    fill_step()

                # normalization for this (pair, q window)
                sums = npool.tile([1, 1024], F32, tag="sums")
                for h01 in range(2):
                    nc.vector.tensor_copy(
                        sums[0:1, h01 * 512:(h01 + 1) * 512],
                        av[h01][64:65, :])
                nc.vector.reciprocal_approx_fast(sums[:], sums[:])
                bc = npool.tile([64, 1024], F32, tag="bc")
                nc.gpsimd.partition_broadcast(bc[:], sums[0:1, :], channels=64)
                tmp = npool.tile([P, 512], F32, tag="tmp")
                for h01 in range(2):
                    nc.vector.tensor_tensor(
                        out=tmp[h01 * 64:(h01 + 1) * 64, :],
                        in0=av[h01][0:64, :],
                        in1=bc[:, h01 * 512:(h01 + 1) * 512],
                        op=ALU.mult)
                nc.vector.tensor_scalar(
                    out=out_norm[p_][:, qlo:qlo + 512], in0=tmp[:],
                    scalar1=bv_t[:, p_:p_ + 1], scalar2=None, op0=ALU.add)

            # ---- schedule ----
            # proj(0) runs immediately; attn(qc) for window qc only needs
            # chunks 0..qc, so attn(qc) is filled with proj(qc+1) and, in
            # later stretches, outproj units for already-final q tiles.
            units = [proj_units(c) for c in range(NSC)]
            for u in units[0]:
                u()
            for qc in range(NSC):
                if qc + 1 < NSC:
                    fill_q.extend(units[qc + 1])
                if qc == 2:
                    fill_q.extend(
                        lambda qt=qt: outproj_unit(qt) for qt in range(4))
                elif qc == 3:
                    fill_q.extend(
                        lambda qt=qt: outproj_unit(qt) for qt in range(4, 12))
                debt[0] = 0.0
                for p_ in range(NPAIR):
                    attn(p_, qc)
                if qc + 1 < NSC:
                    # chunk qc+1 must be fully projected before attn(qc+1)
                    while fill_q:
                        fill_q.popleft()()
            while fill_q:
                fill_q.popleft()()
            for qt in range(12, NQT):
                outproj_unit(qt)
    nc.finalize()
    return nc


_NC_CACHE = None


def _get_nc():
    global _NC_CACHE
    if _NC_CACHE is None:
        _NC_CACHE = _build_nc()
    return _NC_CACHE


def _bf(x):
    return np.ascontiguousarray(np.asarray(x, np.float32)).astype(
        ml_dtypes.bfloat16)


def _prepare_in_maps(query, key, value, Wq, bq, Wk, bk, Wv, bv, Wo):
    qTr = [_bf(query[b].T) for b in range(B)]
    kTr = [_bf(key[b].T) for b in range(B)]
    vTr = [_bf(value[b].T) for b in range(B)]

    def wslice(Wx, g):
        return _bf(Wx[g * HPC:(g + 1) * HPC, :].T)

    def bslice(bx, g):
        return np.ascontiguousarray(
            np.asarray(bx, np.float32)[g * HPC:(g + 1) * HPC]
            .reshape(NPAIR, P).T)

    wq_g = [wslice(Wq, g) for g in range(2)]
    wk_g = [wslice(Wk, g) for g in range(2)]
    wv_g = [wslice(Wv, g) for g in range(2)]
    wo_g = [_bf(np.asarray(Wo, np.float32)[:, g * HPC:(g + 1) * HPC].T)
            for g in range(2)]
    bq_g = [bslice(bq, g) for g in range(2)]
    bk_g = [bslice(bk, g) for g in range(2)]
    bv_g = [bslice(bv, g) for g in range(2)]

    in_maps = []
    for c in range(NCORES):
        b, g = c // 2, c % 2
        in_maps.append({
            "qTr": qTr[b], "kTr": kTr[b], "vTr": vTr[b],
            "wq": wq_g[g], "wk": wk_g[g], "wv": wv_g[g], "wo": wo_g[g],
            "bq": bq_g[g], "bk": bk_g[g], "bv": bv_g[g],
        })
    return in_maps


def kernel(query, key, value, attn_mask, Wq, bq, Wk, bk, Wv, bv, Wo, bo,
           _want_timing=False):
    in_maps = _prepare_in_maps(query, key, value, Wq, bq, Wk, bk, Wv, bv, Wo)
    nc = _get_nc()
    res = run_bass_kernel_spmd(nc, in_maps, list(range(NCORES)),
                               trace=bool(_want_timing))
    bo = np.asarray(bo, np.float32)
    out = np.empty((B, S, E), np.float32)
    for b in range(B):
        out[b] = res.results[2 * b]["out"] + res.results[2 * b + 1]["out"] + bo
    if _want_timing:
        return out, res
    return out


# revision 13
# speedup vs baseline: 1.3249x; 1.0485x over previous
"""Multihead causal attention on 8 TRN2 NeuronCores.

Problem: B=4, S=2048, E=1024, H=16 heads, D=64. Causal mask, eval mode.
Sharding: batch x head-group. Core c -> batch b = c//2, head group g = c%2
(8 heads = 512 hidden dims per core). Each core computes QKV projections for
its head group on its batch, causal attention, and a partial output
projection. Host sums the two partials per batch and adds bo.

v3 layout/schedule strategy (per core):
  - All inputs host-packed so each (tensor, chunk) is ONE contiguous 2D DMA.
  - q/k projections in [d, s] layout (qT/kT [128, 2048] per head pair) so the
    scores matmul scoresT[k, q] = kT.T @ qT needs no on-chip transposes.
  - Attention processed per (pair, 512-wide q window): both heads' scores go
    into one [128, 1024] fp32 PSUM tile (2 banks; the pair's K=64 matmuls at
    base partitions 0/64 run in concurrent PE row groups), one wide exp
    ACTIVATE per k-tile covers both heads.
  - Causal trimming at 128-col granularity; sub-diagonal cols never read.
  - v stored per pair as [128, 16*130] with ones at cols 64/129 of each
    130-block -> AV matmul yields softmax denominators for free (row 64).
  - Projection (per 512-chunk) and output-projection work is chunked into
    ~2us "fill units" interleaved into the attention instruction stream to
    keep the PE busy while ScalarE (exp) limits the attention dataflow.
  - No max-subtraction in softmax: scores ~ N(0,1) by construction.
"""
import math
from collections import deque
import numpy as np
import ml_dtypes

import concourse.bass as bass
import concourse.mybir as mybir
import concourse.tile as tile
from concourse import bacc
from concourse.bass_utils import run_bass_kernel_spmd

F32 = mybir.dt.float32
BF16 = mybir.dt.bfloat16
AF = mybir.ActivationFunctionType
ALU = mybir.AluOpType

B, S, E, H, D = 4, 2048, 1024, 16, 64
P = 128
NCORES = 8
HPC = 512          # hidden dims per core (8 heads)
NPAIR = 4          # head pairs per core
NSC = S // 512     # 4 s-chunks of 512
NST = S // P       # 16 s-tiles of 128
NQT = S // P       # 16 q-tiles for out proj
NE = E // P        # 8 e-chunks
ACHUNK = NE * 512  # 4096 free cols per activation chunk


def _build_nc(debug=False):
    nc = bacc.Bacc(None)
    # host-packed layouts (see _prepare_in_maps):
    #   qTr/kTr/vTr: [128, NSC*NE*512]  chunk-major, then e-major
    #   wq/wk/wv:    [128, NE*512]      e-major ([:, e*512:..] = rows e*128..)
    #   wo:          [128, NPAIR*E]     pair-major
    qTr = nc.declare_dram_parameter("qTr", [P, NSC * ACHUNK], BF16, isOutput=False)
    kTr = nc.declare_dram_parameter("kTr", [P, NSC * ACHUNK], BF16, isOutput=False)
    vTr = nc.declare_dram_parameter("vTr", [P, NSC * ACHUNK], BF16, isOutput=False)
    wq = nc.declare_dram_parameter("wq", [P, NE * HPC], BF16, isOutput=False)
    wk = nc.declare_dram_parameter("wk", [P, NE * HPC], BF16, isOutput=False)
    wv = nc.declare_dram_parameter("wv", [P, NE * HPC], BF16, isOutput=False)
    wo = nc.declare_dram_parameter("wo", [P, NPAIR * E], BF16, isOutput=False)
    bq = nc.declare_dram_parameter("bq", [P, NPAIR], F32, isOutput=False)
    bk = nc.declare_dram_parameter("bk", [P, NPAIR], F32, isOutput=False)
    bv = nc.declare_dram_parameter("bv", [P, NPAIR], F32, isOutput=False)
    out = nc.declare_dram_parameter("out", [S, E], F32, isOutput=True)

    with tile.TileContext(nc) as tc:
        with (
            tc.tile_pool(name="persist", bufs=1) as persist,
            tc.tile_pool(name="weights", bufs=1) as wpool,
            tc.tile_pool(name="acts", bufs=2) as apool,
            tc.tile_pool(name="ps512", bufs=2, space="PSUM") as ps512,
            tc.tile_pool(name="sc_ps", bufs=2, space="PSUM") as sc_pool,
            tc.tile_pool(name="av_ps", bufs=2, space="PSUM") as av_pool,
            tc.tile_pool(name="exp", bufs=4) as ex_pool,
            tc.tile_pool(name="norm", bufs=1) as npool,
            tc.tile_pool(name="fin", bufs=3) as fin_pool,
        ):
            # ---- persistent tiles ----
            qT = [persist.tile([P, S], BF16, tag=f"qT{p}", name=f"qT{p}")
                  for p in range(NPAIR)]
            kT = [persist.tile([P, S], BF16, tag=f"kT{p}", name=f"kT{p}")
                  for p in range(NPAIR)]
            # v per pair: 16 blocks of 130 cols; cols 64/129 of each block = 1
            v_aug = [persist.tile([P, NST * 130], BF16, tag=f"v{p}",
                                  name=f"v{p}") for p in range(NPAIR)]
            out_norm = [persist.tile([P, S], BF16, tag=f"on{p}", name=f"on{p}")
                        for p in range(NPAIR)]
            bq_t = persist.tile([P, NPAIR], F32, tag="bq")
            bk_t = persist.tile([P, NPAIR], F32, tag="bk")
            bv_t = persist.tile([P, NPAIR], F32, tag="bv")
            nc.sync.dma_start(out=bq_t[:], in_=bq[:, :])
            nc.sync.dma_start(out=bk_t[:], in_=bk[:, :])
            nc.sync.dma_start(out=bv_t[:], in_=bv[:, :])

            # ones columns of v_aug: strided memset (offsets 64+65k)
            for p_ in range(NPAIR):
                nc.gpsimd.memset(v_aug[p_][:, 64:NST * 130:65], 1.0)

            # ---- weights: one DMA each ----
            wq_a = wpool.tile([P, NE * HPC], BF16, tag="wq", name="wq")
            wk_a = wpool.tile([P, NE * HPC], BF16, tag="wk", name="wk")
            wv_a = wpool.tile([P, NE * HPC], BF16, tag="wv", name="wv")
            wo_a = wpool.tile([P, NPAIR * E], BF16, tag="wo", name="wo")
            nc.sync.dma_start(out=wq_a[:], in_=wq[:, :])
            nc.sync.dma_start(out=wk_a[:], in_=wk[:, :])
            nc.sync.dma_start(out=wv_a[:], in_=wv[:, :])
            nc.sync.dma_start(out=wo_a[:], in_=wo[:, :])

            # ---- projection work, chunked into fill units ----
            def proj_dma(c):
                tiles = {}
                for nm, src in (("q", qTr), ("k", kTr), ("v", vTr)):
                    t = apool.tile([P, ACHUNK], BF16, tag=f"a{nm}",
                                   name=f"{nm}tr{c}")
                    nc.sync.dma_start(
                        out=t[:], in_=src[:, c * ACHUNK:(c + 1) * ACHUNK])
                    tiles[nm] = t
                return tiles

            def proj_qk_unit(c, acts, m, which):
                # one 512-col projection accumulator for pair m (q or k)
                cs = slice(c * 512, (c + 1) * 512)
                w_a, b_t, dstT, nm = {
                    "q": (wq_a, bq_t, qT, "q"), "k": (wk_a, bk_t, kT, "k"),
                }[which]
                ps = ps512.tile([P, 512], F32, tag="ps512", name=f"{nm}ps{c}_{m}")
                for e in range(NE):
                    nc.tensor.matmul(
                        ps[:],
                        w_a[:, e * HPC + m * P:e * HPC + (m + 1) * P],
                        acts[nm][:, e * 512:(e + 1) * 512],
                        start=(e == 0), stop=(e == NE - 1))
                nc.vector.tensor_scalar(
                    out=dstT[m][:, cs], in0=ps[:],
                    scalar1=b_t[:, m:m + 1], scalar2=None, op0=ALU.add)

            def proj_v_unit(c, acts, i):
                # v projection for s-tile i of chunk c: out [s128, 512 hd]
                st = c * 4 + i
                ps = ps512.tile([P, 512], F32, tag="ps512", name=f"vps{c}_{i}")
                for e in range(NE):
                    nc.tensor.matmul(
                        ps[:],
                        acts["v"][:, e * 512 + i * P:e * 512 + (i + 1) * P],
                        wv_a[:, e * HPC:(e + 1) * HPC],
                        start=(e == 0), stop=(e == NE - 1))
                for p_ in range(NPAIR):
                    base = st * 130
                    nc.vector.tensor_copy(
                        v_aug[p_][:, base:base + 64],
                        ps[:, p_ * P:p_ * P + 64])
                    nc.vector.tensor_copy(
                        v_aug[p_][:, base + 65:base + 129],
                        ps[:, p_ * P + 64:(p_ + 1) * P])

            def proj_units(c):
                units = []
                acts = {}
                def dma_unit():
                    acts.update(proj_dma(c))
                units.append(dma_unit)
                for m in range(NPAIR):
                    units.append(lambda m=m: proj_qk_unit(c, acts, m, "q"))
                    units.append(lambda m=m: proj_qk_unit(c, acts, m, "k"))
                for i in range(4):
                    units.append(lambda i=i: proj_v_unit(c, acts, i))
                return units

            # ---- output projection unit (one q-tile) ----
            def outproj_unit(qt):
                fin = fin_pool.tile([P, E], F32, tag="fin")
                for ec in range(2):
                    ops = ps512.tile([P, 512], F32, tag="ps512", name=f"o{qt}_{ec}")
                    for p_ in range(NPAIR):
                        nc.tensor.matmul(
                            ops[:],
                            out_norm[p_][:, qt * P:(qt + 1) * P],
                            wo_a[:, p_ * E + ec * 512:p_ * E + (ec + 1) * 512],
                            start=(p_ == 0), stop=(p_ == NPAIR - 1))
                    nc.vector.tensor_copy(fin[:, ec * 512:(ec + 1) * 512], ops[:])
                nc.sync.dma_start(out=out[qt * P:(qt + 1) * P, :], in_=fin[:])

            # ---- attention for one (pair, q window), with PE fill units ----
            fill_q = deque()
            debt = [0.0]

            def fill_step():
                while debt[0] > 2000.0 and fill_q:
                    fill_q.popleft()()
                    debt[0] -= 2000.0

            def attn(p_, qc):
                # one pair, one 512-wide q window [qc*512, (qc+1)*512)
                qlo = qc * 512
                nkt = 4 * qc + 4
                av = [av_pool.tile([65, 512], F32, tag="av", name=f"av{h}")
                      for h in range(2)]
                for kt in range(nkt):
                    k0 = kt * P
                    sstart = max(0, k0 - qlo)
                    diag = kt >= 4 * qc
                    # both heads' scores in one 2-bank tile (h01 at 512*h01)
                    sc = sc_pool.tile([P, 1024], F32, tag="sc", name="sc")
                    for h01 in range(2):
                        hsl = slice(h01 * 64, (h01 + 1) * 64)
                        nc.tensor.matmul(
                            sc[:, h01 * 512 + sstart:(h01 + 1) * 512],
                            kT[p_][hsl, k0:k0 + P],
                            qT[p_][hsl, qlo + sstart:qlo + 512],
                            start=True, stop=True)
                    ex = ex_pool.tile([P, 1024], BF16, tag="ex", name="ex")
                    nc.scalar.activation(
                        ex[:, sstart:1024], sc[:, sstart:1024],
                        AF.Exp, scale=1.0 / math.sqrt(D))
                    if diag:
                        for h01 in range(2):
                            o = h01 * 512 + sstart
                            nc.gpsimd.affine_select(
                                out=ex[:, o:o + P], in_=ex[:, o:o + P],
                                compare_op=ALU.is_ge, fill=0.0,
                                base=0, channel_multiplier=-1,
                                pattern=[[1, P]])
                    for h01 in range(2):
                        vbase = kt * 130 + h01 * 65
                        nc.tensor.matmul(
                            av[h01][:, sstart:512],
                            v_aug[p_][:, vbase:vbase + 65],
                            ex[:, h01 * 512 + sstart:(h01 + 1) * 512],
                            start=(kt == 0), stop=(kt == nkt - 1))
                    # PE/Scalar balance bookkeeping -> drain fill units
                    W = 512 - sstart
                    scalar_ns = (1024 - sstart + 352) / 1.2
                    pe_ns = 3 * W * 0.48
                    debt[0] += scalar_ns - pe_ns
                    fill_step()

                # normalization for this (pair, q window)
                sums = npool.tile([1, 1024], F32, tag="sums")
                for h01 in range(2):
                    nc.vector.tensor_copy(
                        sums[0:1, h01 * 512:(h01 + 1) * 512],
                        av[h01][64:65, :])
                nc.vector.reciprocal_approx_fast(sums[:], sums[:])
                bc = npool.tile([64, 1024], F32, tag="bc")
                nc.gpsimd.partition_broadcast(bc[:], sums[0:1, :], channels=64)
                tmp = npool.tile([P, 512], F32, tag="tmp")
                for h01 in range(2):
                    nc.vector.tensor_tensor(
                        out=tmp[h01 * 64:(h01 + 1) * 64, :],
                        in0=av[h01][0:64, :],
                        in1=bc[:, h01 * 512:(h01 + 1) * 512],
                        op=ALU.mult)
                nc.vector.tensor_scalar(
                    out=out_norm[p_][:, qlo:qlo + 512], in0=tmp[:],
                    scalar1=bv_t[:, p_:p_ + 1], scalar2=None, op0=ALU.add)

            # ---- schedule ----
            # proj(0) runs immediately; attn(qc) for window qc only needs
            # chunks 0..qc, so attn(qc) is filled with proj(qc+1) and, in
            # later stretches, outproj units for already-final q tiles.
            units = [proj_units(c) for c in range(NSC)]
            for u in units[0]:
                u()
            for qc in range(NSC):
                if qc + 1 < NSC:
                    fill_q.extend(units[qc + 1])
                if qc == 2:
                    fill_q.extend(
                        lambda qt=qt: outproj_unit(qt) for qt in range(4))
                elif qc == 3:
                    fill_q.extend(
                        lambda qt=qt: outproj_unit(qt) for qt in range(4, 12))
                debt[0] = 0.0
                for p_ in range(NPAIR):
                    attn(p_, qc)
                if qc + 1 < NSC:
                    # chunk qc+1 must be fully projected before attn(qc+1)
                    while fill_q:
                        fill_q.popleft()()
            while fill_q:
                fill_q.popleft()()
            for qt in range(12, NQT):
                outproj_unit(qt)
    nc.finalize()
    return nc


_NC_CACHE = None


def _get_nc():
    global _NC_CACHE
    if _NC_CACHE is None:
        _NC_CACHE = _build_nc()
    return _NC_CACHE


def _bf(x):
    return np.ascontiguousarray(np.asarray(x, np.float32)).astype(
        ml_dtypes.bfloat16)


def _pack_acts(xT):
    # [E, S] -> [128, NSC*NE*512], chunk-major then e-major
    return np.ascontiguousarray(
        xT.reshape(NE, P, NSC, 512).transpose(1, 2, 0, 3).reshape(P, -1))


def _pack_w(wT):
    # [E, HPC] -> [128, NE*HPC]
    return np.ascontiguousarray(
        wT.reshape(NE, P, HPC).transpose(1, 0, 2).reshape(P, -1))


def _pack_wo(woT):
    # [HPC, E] -> [128, NPAIR*E]
    return np.ascontiguousarray(
        woT.reshape(NPAIR, P, E).transpose(1, 0, 2).reshape(P, -1))


def _prepare_in_maps(query, key, value, Wq, bq, Wk, bk, Wv, bv, Wo):
    qTr = [_pack_acts(_bf(query[b].T)) for b in range(B)]
    kTr = [_pack_acts(_bf(key[b].T)) for b in range(B)]
    vTr = [_pack_acts(_bf(value[b].T)) for b in range(B)]

    def wslice(Wx, g):
        return _pack_w(_bf(Wx[g * HPC:(g + 1) * HPC, :].T))

    def bslice(bx, g):
        return np.ascontiguousarray(
            np.asarray(bx, np.float32)[g * HPC:(g + 1) * HPC]
            .reshape(NPAIR, P).T)

    wq_g = [wslice(Wq, g) for g in range(2)]
    wk_g = [wslice(Wk, g) for g in range(2)]
    wv_g = [wslice(Wv, g) for g in range(2)]
    wo_g = [_pack_wo(_bf(np.asarray(Wo, np.float32)[:, g * HPC:(g + 1) * HPC].T))
            for g in range(2)]
    bq_g = [bslice(bq, g) for g in range(2)]
    bk_g = [bslice(bk, g) for g in range(2)]
    bv_g = [bslice(bv, g) for g in range(2)]

    in_maps = []
    for c in range(NCORES):
        b, g = c // 2, c % 2
        in_maps.append({
            "qTr": qTr[b], "kTr": kTr[b], "vTr": vTr[b],
            "wq": wq_g[g], "wk": wk_g[g], "wv": wv_g[g], "wo": wo_g[g],
            "bq": bq_g[g], "bk": bk_g[g], "bv": bv_g[g],
        })
    return in_maps


def kernel(query, key, value, attn_mask, Wq, bq, Wk, bk, Wv, bv, Wo, bo,
           _want_timing=False):
    in_maps = _prepare_in_maps(query, key, value, Wq, bq, Wk, bk, Wv, bv, Wo)
    nc = _get_nc()
    res = run_bass_kernel_spmd(nc, in_maps, list(range(NCORES)),
                               trace=bool(_want_timing))
    bo = np.asarray(bo, np.float32)
    out = np.empty((B, S, E), np.float32)
    for b in range(B):
        out[b] = res.results[2 * b]["out"] + res.results[2 * b + 1]["out"] + bo
    if _want_timing:
        return out, res
    return out


# revision 17
# speedup vs baseline: 1.3768x; 1.0391x over previous
"""Multihead causal attention on 8 TRN2 NeuronCores.

Problem: B=4, S=2048, E=1024, H=16 heads, D=64. Causal mask, eval mode.
Sharding: batch x head-group. Core c -> batch b = c//2, head group g = c%2
(8 heads = 512 hidden dims per core). Each core computes QKV projections for
its head group on its batch, causal attention, and a partial output
projection. Host sums the two partials per batch and adds bo.

v3 layout/schedule strategy (per core):
  - All inputs host-packed so each (tensor, chunk) is ONE contiguous 2D DMA.
  - q/k projections in [d, s] layout (qT/kT [128, 2048] per head pair) so the
    scores matmul scoresT[k, q] = kT.T @ qT needs no on-chip transposes.
  - Attention processed per (pair, 512-wide q window): both heads' scores go
    into one [128, 1024] fp32 PSUM tile (2 banks; the pair's K=64 matmuls at
    base partitions 0/64 run in concurrent PE row groups), one wide exp
    ACTIVATE per k-tile covers both heads.
  - Causal trimming at 128-col granularity; sub-diagonal cols never read.
  - v stored per pair as [128, 16*130] with ones at cols 64/129 of each
    130-block -> AV matmul yields softmax denominators for free (row 64).
  - Projection (per 512-chunk) and output-projection work is chunked into
    ~2us "fill units" interleaved into the attention instruction stream to
    keep the PE busy while ScalarE (exp) limits the attention dataflow.
  - No max-subtraction in softmax: scores ~ N(0,1) by construction.
"""
import math
from collections import deque
import numpy as np
import ml_dtypes

import concourse.bass as bass
import concourse.mybir as mybir
import concourse.tile as tile
from concourse import bacc
from concourse.bass_utils import run_bass_kernel_spmd

F32 = mybir.dt.float32
BF16 = mybir.dt.bfloat16
AF = mybir.ActivationFunctionType
ALU = mybir.AluOpType

B, S, E, H, D = 4, 2048, 1024, 16, 64
P = 128
NCORES = 8
HPC = 512          # hidden dims per core (8 heads)
NPAIR = 4          # head pairs per core
NSC = S // 512     # 4 s-chunks of 512
NST = S // P       # 16 s-tiles of 128
NQT = S // P       # 16 q-tiles for out proj
NE = E // P        # 8 e-chunks
ACHUNK = NE * 512  # 4096 free cols per activation chunk


def _build_nc(debug=False):
    nc = bacc.Bacc(None)
    # host-packed layouts (see _prepare_in_maps):
    #   qTr/kTr/vTr: [128, NSC*NE*512]  chunk-major, then e-major
    #   wq/wk/wv:    [128, NE*512]      e-major ([:, e*512:..] = rows e*128..)
    #   wo:          [128, NPAIR*E]     pair-major
    qTr = nc.declare_dram_parameter("qTr", [P, NSC * ACHUNK], BF16, isOutput=False)
    kTr = nc.declare_dram_parameter("kTr", [P, NSC * ACHUNK], BF16, isOutput=False)
    vTr = nc.declare_dram_parameter("vTr", [P, NSC * ACHUNK], BF16, isOutput=False)
    wq = nc.declare_dram_parameter("wq", [P, NE * HPC], BF16, isOutput=False)
    wk = nc.declare_dram_parameter("wk", [P, NE * HPC], BF16, isOutput=False)
    wv = nc.declare_dram_parameter("wv", [P, NE * HPC], BF16, isOutput=False)
    wo = nc.declare_dram_parameter("wo", [P, NPAIR * E], BF16, isOutput=False)
    bq = nc.declare_dram_parameter("bq", [P, NPAIR], F32, isOutput=False)
    bk = nc.declare_dram_parameter("bk", [P, NPAIR], F32, isOutput=False)
    bv = nc.declare_dram_parameter("bv", [P, NPAIR], F32, isOutput=False)
    out = nc.declare_dram_parameter("out", [S, E], F32, isOutput=True)

    with tile.TileContext(nc) as tc:
        with (
            tc.tile_pool(name="persist", bufs=1) as persist,
            tc.tile_pool(name="weights", bufs=1) as wpool,
            tc.tile_pool(name="acts", bufs=2) as apool,
            tc.tile_pool(name="ps512", bufs=2, space="PSUM") as ps512,
            tc.tile_pool(name="sc_ps", bufs=2, space="PSUM") as sc_pool,
            tc.tile_pool(name="av_ps", bufs=2, space="PSUM") as av_pool,
            tc.tile_pool(name="exp", bufs=4) as ex_pool,
            tc.tile_pool(name="norm", bufs=2) as npool,
            tc.tile_pool(name="fin", bufs=3) as fin_pool,
        ):
            # ---- persistent tiles ----
            qT = [persist.tile([P, S], BF16, tag=f"qT{p}", name=f"qT{p}")
                  for p in range(NPAIR)]
            kT = [persist.tile([P, S], BF16, tag=f"kT{p}", name=f"kT{p}")
                  for p in range(NPAIR)]
            # v per pair: 16 blocks of 130 cols; cols 64/129 of each block = 1
            v_aug = [persist.tile([P, NST * 130], BF16, tag=f"v{p}",
                                  name=f"v{p}") for p in range(NPAIR)]
            out_norm = [persist.tile([P, S], BF16, tag=f"on{p}", name=f"on{p}")
                        for p in range(NPAIR)]
            bq_t = persist.tile([P, NPAIR], F32, tag="bq")
            bk_t = persist.tile([P, NPAIR], F32, tag="bk")
            bv_t = persist.tile([P, NPAIR], F32, tag="bv")
            nc.sync.dma_start(out=bq_t[:], in_=bq[:, :])
            nc.sync.dma_start(out=bk_t[:], in_=bk[:, :])
            nc.sync.dma_start(out=bv_t[:], in_=bv[:, :])

            # ones columns of v_aug: strided memset (offsets 64+65k)
            for p_ in range(NPAIR):
                nc.gpsimd.memset(v_aug[p_][:, 64:NST * 130:65], 1.0)

            # ---- weights: one DMA each ----
            wq_a = wpool.tile([P, NE * HPC], BF16, tag="wq", name="wq")
            wk_a = wpool.tile([P, NE * HPC], BF16, tag="wk", name="wk")
            wv_a = wpool.tile([P, NE * HPC], BF16, tag="wv", name="wv")
            wo_a = wpool.tile([P, NPAIR * E], BF16, tag="wo", name="wo")
            # DMA queue is serial: order by first use (wq -> chunk0 q acts
            # -> wk/ak -> wv/av -> wo). Chunk-0 act DMAs are issued here,
            # between the weight DMAs, and handed to proj_units(0) below.
            acts0 = {}
            nc.sync.dma_start(out=wq_a[:], in_=wq[:, :])
            for nm, src in (("q", qTr), ("k", kTr), ("v", vTr)):
                t = apool.tile([P, ACHUNK], BF16, tag=f"a{nm}", name=f"{nm}tr0")
                nc.sync.dma_start(out=t[:], in_=src[:, 0:ACHUNK])
                acts0[nm] = t
                if nm == "q":
                    nc.sync.dma_start(out=wk_a[:], in_=wk[:, :])
                elif nm == "k":
                    nc.sync.dma_start(out=wv_a[:], in_=wv[:, :])
            nc.sync.dma_start(out=wo_a[:], in_=wo[:, :])

            # ---- projection work, chunked into fill units ----
            def proj_dma(c):
                tiles = {}
                for nm, src in (("q", qTr), ("k", kTr), ("v", vTr)):
                    t = apool.tile([P, ACHUNK], BF16, tag=f"a{nm}",
                                   name=f"{nm}tr{c}")
                    nc.sync.dma_start(
                        out=t[:], in_=src[:, c * ACHUNK:(c + 1) * ACHUNK])
                    tiles[nm] = t
                return tiles

            def proj_qk_unit(c, acts, m, which):
                # one 512-col projection accumulator for pair m (q or k)
                cs = slice(c * 512, (c + 1) * 512)
                w_a, b_t, dstT, nm = {
                    "q": (wq_a, bq_t, qT, "q"), "k": (wk_a, bk_t, kT, "k"),
                }[which]
                ps = ps512.tile([P, 512], F32, tag="ps512", name=f"{nm}ps{c}_{m}")
                for e in range(NE):
                    nc.tensor.matmul(
                        ps[:],
                        w_a[:, e * HPC + m * P:e * HPC + (m + 1) * P],
                        acts[nm][:, e * 512:(e + 1) * 512],
                        start=(e == 0), stop=(e == NE - 1))
                nc.vector.tensor_scalar(
                    out=dstT[m][:, cs], in0=ps[:],
                    scalar1=b_t[:, m:m + 1], scalar2=None, op0=ALU.add)

            def proj_v_unit(c, acts, i):
                # v projection for s-tile i of chunk c: out [s128, 512 hd]
                st = c * 4 + i
                ps = ps512.tile([P, 512], F32, tag="ps512", name=f"vps{c}_{i}")
                for e in range(NE):
                    nc.tensor.matmul(
                        ps[:],
                        acts["v"][:, e * 512 + i * P:e * 512 + (i + 1) * P],
                        wv_a[:, e * HPC:(e + 1) * HPC],
                        start=(e == 0), stop=(e == NE - 1))
                for p_ in range(NPAIR):
                    base = st * 130
                    nc.vector.tensor_copy(
                        v_aug[p_][:, base:base + 64],
                        ps[:, p_ * P:p_ * P + 64])
                    nc.vector.tensor_copy(
                        v_aug[p_][:, base + 65:base + 129],
                        ps[:, p_ * P + 64:(p_ + 1) * P])

            def proj_units(c, preloaded=None):
                units = []
                acts = {}
                if preloaded is not None:
                    acts.update(preloaded)
                else:
                    def dma_unit():
                        acts.update(proj_dma(c))
                    units.append(dma_unit)
                for m in range(NPAIR):
                    units.append(lambda m=m: proj_qk_unit(c, acts, m, "q"))
                    units.append(lambda m=m: proj_qk_unit(c, acts, m, "k"))
                for i in range(4):
                    units.append(lambda i=i: proj_v_unit(c, acts, i))
                return units

            # ---- output projection unit (one q-tile) ----
            def outproj_unit(qt):
                fin = fin_pool.tile([P, E], F32, tag="fin")
                for ec in range(2):
                    ops = ps512.tile([P, 512], F32, tag="ps512", name=f"o{qt}_{ec}")
                    for p_ in range(NPAIR):
                        nc.tensor.matmul(
                            ops[:],
                            out_norm[p_][:, qt * P:(qt + 1) * P],
                            wo_a[:, p_ * E + ec * 512:p_ * E + (ec + 1) * 512],
                            start=(p_ == 0), stop=(p_ == NPAIR - 1))
                    nc.vector.tensor_copy(fin[:, ec * 512:(ec + 1) * 512], ops[:])
                nc.sync.dma_start(out=out[qt * P:(qt + 1) * P, :], in_=fin[:])

            # ---- attention for one (pair, q window), with PE fill units ----
            fill_q = deque()
            debt = [0.0]

            def fill_step():
                while debt[0] > 2000.0 and fill_q:
                    fill_q.popleft()()
                    debt[0] -= 2000.0

            def attn(p_, qc):
                # one pair, one 512-wide q window [qc*512, (qc+1)*512)
                qlo = qc * 512
                nkt = 4 * qc + 4
                av = [av_pool.tile([65, 512], F32, tag="av", name=f"av{h}")
                      for h in range(2)]
                for kt in range(nkt):
                    k0 = kt * P
                    sstart = max(0, k0 - qlo)
                    diag = kt >= 4 * qc
                    # both heads' scores in one 2-bank tile (h01 at 512*h01)
                    sc = sc_pool.tile([P, 1024], F32, tag="sc", name="sc")
                    for h01 in range(2):
                        hsl = slice(h01 * 64, (h01 + 1) * 64)
                        nc.tensor.matmul(
                            sc[:, h01 * 512 + sstart:(h01 + 1) * 512],
                            kT[p_][hsl, k0:k0 + P],
                            qT[p_][hsl, qlo + sstart:qlo + 512],
                            start=True, stop=True)
                    ex = ex_pool.tile([P, 1024], BF16, tag="ex", name="ex")
                    nc.scalar.activation(
                        ex[:, sstart:1024], sc[:, sstart:1024],
                        AF.Exp, scale=1.0 / math.sqrt(D))
                    if diag:
                        for h01 in range(2):
                            o = h01 * 512 + sstart
                            nc.gpsimd.affine_select(
                                out=ex[:, o:o + P], in_=ex[:, o:o + P],
                                compare_op=ALU.is_ge, fill=0.0,
                                base=0, channel_multiplier=-1,
                                pattern=[[1, P]])
                    for h01 in range(2):
                        vbase = kt * 130 + h01 * 65
                        nc.tensor.matmul(
                            av[h01][:, sstart:512],
                            v_aug[p_][:, vbase:vbase + 65],
                            ex[:, h01 * 512 + sstart:(h01 + 1) * 512],
                            start=(kt == 0), stop=(kt == nkt - 1))
                    # PE/Scalar balance bookkeeping -> drain fill units
                    W = 512 - sstart
                    scalar_ns = (1024 - sstart + 352) / 1.2
                    pe_ns = 3 * W * 0.48
                    debt[0] += scalar_ns - pe_ns
                    fill_step()

                # normalization for this (pair, q window)
                sums = npool.tile([1, 1024], F32, tag="sums")
                for h01 in range(2):
                    nc.vector.tensor_copy(
                        sums[0:1, h01 * 512:(h01 + 1) * 512],
                        av[h01][64:65, :])
                nc.vector.reciprocal_approx_fast(sums[:], sums[:])
                bc = npool.tile([64, 1024], F32, tag="bc")
                nc.gpsimd.partition_broadcast(bc[:], sums[0:1, :], channels=64)
                tmp = npool.tile([P, 512], F32, tag="tmp")
                for h01 in range(2):
                    nc.vector.tensor_tensor(
                        out=tmp[h01 * 64:(h01 + 1) * 64, :],
                        in0=av[h01][0:64, :],
                        in1=bc[:, h01 * 512:(h01 + 1) * 512],
                        op=ALU.mult)
                nc.vector.tensor_scalar(
                    out=out_norm[p_][:, qlo:qlo + 512], in0=tmp[:],
                    scalar1=bv_t[:, p_:p_ + 1], scalar2=None, op0=ALU.add)

            # ---- schedule ----
            # proj(0) runs immediately; attn(qc) for window qc only needs
            # chunks 0..qc, so attn(qc) is filled with proj(qc+1) and, in
            # later stretches, outproj units for already-final q tiles.
            units = [proj_units(0, preloaded=acts0)] + [
                proj_units(c) for c in range(1, NSC)]
            for u in units[0]:
                u()
            for qc in range(NSC):
                if qc + 1 < NSC:
                    fill_q.extend(units[qc + 1])
                if qc == 2:
                    fill_q.extend(
                        lambda qt=qt: outproj_unit(qt) for qt in range(4))
                elif qc == 3:
                    fill_q.extend(
                        lambda qt=qt: outproj_unit(qt) for qt in range(4, 12))
                debt[0] = 0.0
                for p_ in range(NPAIR):
                    attn(p_, qc)
                if qc + 1 < NSC:
                    # chunk qc+1 must be fully projected before attn(qc+1)
                    while fill_q:
                        fill_q.popleft()()
            while fill_q:
                fill_q.popleft()()
            for qt in range(12, NQT):
                outproj_unit(qt)
    nc.finalize()
    return nc


_NC_CACHE = None


def _get_nc():
    global _NC_CACHE
    if _NC_CACHE is None:
        _NC_CACHE = _build_nc()
    return _NC_CACHE


def _bf(x):
    return np.ascontiguousarray(np.asarray(x, np.float32)).astype(
        ml_dtypes.bfloat16)


def _pack_acts(xT):
    # [E, S] -> [128, NSC*NE*512], chunk-major then e-major
    return np.ascontiguousarray(
        xT.reshape(NE, P, NSC, 512).transpose(1, 2, 0, 3).reshape(P, -1))


def _pack_w(wT):
    # [E, HPC] -> [128, NE*HPC]
    return np.ascontiguousarray(
        wT.reshape(NE, P, HPC).transpose(1, 0, 2).reshape(P, -1))


def _pack_wo(woT):
    # [HPC, E] -> [128, NPAIR*E]
    return np.ascontiguousarray(
        woT.reshape(NPAIR, P, E).transpose(1, 0, 2).reshape(P, -1))


def _prepare_in_maps(query, key, value, Wq, bq, Wk, bk, Wv, bv, Wo):
    qTr = [_pack_acts(_bf(query[b].T)) for b in range(B)]
    kTr = [_pack_acts(_bf(key[b].T)) for b in range(B)]
    vTr = [_pack_acts(_bf(value[b].T)) for b in range(B)]

    def wslice(Wx, g):
        return _pack_w(_bf(Wx[g * HPC:(g + 1) * HPC, :].T))

    def bslice(bx, g):
        return np.ascontiguousarray(
            np.asarray(bx, np.float32)[g * HPC:(g + 1) * HPC]
            .reshape(NPAIR, P).T)

    wq_g = [wslice(Wq, g) for g in range(2)]
    wk_g = [wslice(Wk, g) for g in range(2)]
    wv_g = [wslice(Wv, g) for g in range(2)]
    wo_g = [_pack_wo(_bf(np.asarray(Wo, np.float32)[:, g * HPC:(g + 1) * HPC].T))
            for g in range(2)]
    bq_g = [bslice(bq, g) for g in range(2)]
    bk_g = [bslice(bk, g) for g in range(2)]
    bv_g = [bslice(bv, g) for g in range(2)]

    in_maps = []
    for c in range(NCORES):
        b, g = c // 2, c % 2
        in_maps.append({
            "qTr": qTr[b], "kTr": kTr[b], "vTr": vTr[b],
            "wq": wq_g[g], "wk": wk_g[g], "wv": wv_g[g], "wo": wo_g[g],
            "bq": bq_g[g], "bk": bk_g[g], "bv": bv_g[g],
        })
    return in_maps


def kernel(query, key, value, attn_mask, Wq, bq, Wk, bk, Wv, bv, Wo, bo,
           _want_timing=False):
    in_maps = _prepare_in_maps(query, key, value, Wq, bq, Wk, bk, Wv, bv, Wo)
    nc = _get_nc()
    res = run_bass_kernel_spmd(nc, in_maps, list(range(NCORES)),
                               trace=bool(_want_timing))
    bo = np.asarray(bo, np.float32)
    out = np.empty((B, S, E), np.float32)
    for b in range(B):
        out[b] = res.results[2 * b]["out"] + res.results[2 * b + 1]["out"] + bo
    if _want_timing:
        return out, res
    return out
